# revision 1
# baseline (speedup 1.0000x reference)
"""Trainium2 Bass kernel for an R-GCN-style GCN layer (basis decomposition).

Reference computation (per relation r, with W_r = sum_b coeff[r,b] * basis[b]):
    out = sum_r segment_sum(inp[src_r] * val_r, dst_r) @ W_r + sum_r bias[r]

Algebraic restructure (4 basis accumulators instead of 16 relation matmuls):
    out[d] = sum_b G_b[d] @ basis[b] + bias_sum
    G_b[d] = sum_{edges e: dst_e = d} (coeff[r_e, b] * val_e) * inp[src_e]

Distribution: output nodes are sharded 8 ways (12500 rows/core); every core
holds the full gather table in its own HBM, so there is no cross-core
communication at all.

Per-core static structure (all shapes identical across cores; only data
differs, as SPMD requires):
  - 100 blocks of 128 dst nodes (98 real), grouped into 25 superblocks (SB)
    of 4 blocks.
  - Edges bucketed by (block, src-segment, group-of-32-dst-nodes). Src is
    split into 4 segments of 25000 so gather indices fit dma_gather's int16.
    Bucket capacity 192 = one K=128 chunk + one K=64 chunk (actual max 174).
  - Per (SB, segment): ONE dma_gather of 3072 rows from a composite table
    with a zero row per segment (padding slots gather zeros).
  - Per chunk: one fused DVE op builds the mask
      M[e, bb*32+n] = (dst_local[e] == n) * coeff[r_e, bb] * val_e
    (K=64 chunk pairs share a single [128,128] mask op), and one PE matmul
    accumulates gT[f, (q, bb, n)] += X_chunk.T @ M into the block's PSUM bank.
  - Per block: 4 basis matmuls outT[fout, n] += basis_b.T @ gT_b, bias fused
    into the PSUM->SBUF copy on the scalar engine.

Output is produced transposed per block ([fout, node]) and reassembled on host.
"""
import os
import sys

for _p in ("/opt/trn_rl_repo", "/root/.axon_site/_ro/trn_rl_repo"):
    if os.path.isdir(_p) and _p not in sys.path:
        sys.path.insert(0, _p)

import numpy as np

import concourse.bass as bass
import concourse.tile as tile
from concourse import bacc, mybir
from concourse.bass_utils import run_bass_kernel_spmd

# ---------------- problem constants (hardcoded from spec) ----------------
NN = 100000          # nodes
F = 128              # feature dim (in == out)
NB = 4               # bases
NREL = 16            # relations
NCORES = 8
NS = NN // NCORES    # dst nodes per core (12500)

GROUP = 32           # dst nodes per group
GPB = 4              # groups per block
BLOCK = GROUP * GPB  # 128 dst nodes per block
NBLK = 100           # padded block count (98 real)
BPS = 4              # blocks per superblock
NSB = NBLK // BPS    # 25 superblocks

NSEG = 4             # src segments
SEG = 25000          # src rows per segment
TBL_ROWS = NN + NSEG # composite table: one zero row per segment

CAP = 192            # bucket capacity: K=128 chunk + K=64 chunk
BUCKETS = BPS * GPB  # 16 buckets per (SB, segment)
CS = BUCKETS + BUCKETS // 2   # 24 X columns per (SB, segment)
SEG_IDX = CS * 128   # 3072 gather rows per (SB, segment)
COLS = NSEG * CS     # 96 X columns per SB

# meta layout per SB (f32): [val: COLS][ldst: COLS][coef: 4*COLS]
META_COLS = 6 * COLS           # 576
IDX_COLS = NSEG * (SEG_IDX // 16)  # 768 int16 cols per SB

F32 = mybir.dt.float32
I16 = mybir.dt.int16

_compiled = {}


def _build_program():
    nc = bacc.Bacc(
        "TRN2",
        target_bir_lowering=False,
        debug=False,
        enable_asserts=False,
        num_devices=NCORES,
    )

    tbl = nc.dram_tensor("tbl", [TBL_ROWS, F], F32, kind="ExternalInput")
    basisw = nc.dram_tensor("basisw", [NB, F, F], F32, kind="ExternalInput")
    biasw = nc.dram_tensor("biasw", [NREL, F], F32, kind="ExternalInput")
    iota = nc.dram_tensor("iota", [128, NB * GROUP], F32, kind="ExternalInput")
    # pair-mask iota: col (qh, bb, n) holds n for partitions with p//64 == qh,
    # else 99 (never matches a dst offset, zeroing the foreign half)
    iota2 = nc.dram_tensor("iota2", [128, 2 * NB * GROUP], F32, kind="ExternalInput")
    eidx = nc.dram_tensor("eidx", [128, NSB * IDX_COLS], I16, kind="ExternalInput")
    meta = nc.dram_tensor("meta", [128, NSB * META_COLS], F32, kind="ExternalInput")
    outT = nc.dram_tensor("outT", [NBLK, F, BLOCK], F32, kind="ExternalOutput")

    with tile.TileContext(nc) as tc:
        with (
            tc.tile_pool(name="const", bufs=1) as const,
            tc.tile_pool(name="xg", bufs=2) as xg,
            tc.tile_pool(name="idxp", bufs=2) as idxp,
            tc.tile_pool(name="metap", bufs=2) as metap,
            tc.tile_pool(name="w4p", bufs=2) as w4p,
            tc.tile_pool(name="msk", bufs=8) as mskp,
            tc.tile_pool(name="gt", bufs=4) as gtp,
            tc.tile_pool(name="ot", bufs=3) as otp,
            tc.tile_pool(name="psg", bufs=5, space="PSUM") as psg,
            tc.tile_pool(name="pso", bufs=2, space="PSUM") as pso,
            tc.tile_pool(name="psb", bufs=1, space="PSUM") as psb,
        ):
            # ---- constants
            iota_t = const.tile([128, NB * GROUP], F32)
            nc.sync.dma_start(out=iota_t[:], in_=iota[:, :])
            iota2_t = const.tile([128, 2 * NB * GROUP], F32)
            nc.sync.dma_start(out=iota2_t[:], in_=iota2[:, :])
            basis_t = const.tile([F, NB * F], F32)
            for b in range(NB):
                nc.sync.dma_start(
                    out=basis_t[:, b * F : (b + 1) * F], in_=basisw[b, :, :]
                )
            bias_sb = const.tile([NREL, F], F32)
            nc.sync.dma_start(out=bias_sb[:], in_=biasw[:, :])
            ones_t = const.tile([NREL, 1], F32)
            nc.vector.memset(ones_t[:], 1.0)
            bias_ps = psb.tile([F, 1], F32)
            nc.tensor.matmul(
                bias_ps[:], lhsT=bias_sb[:], rhs=ones_t[:], start=True, stop=True
            )
            bias_col = const.tile([F, 1], F32)
            nc.scalar.copy(bias_col[:], bias_ps[:])

            for sb in range(NSB):
                idx_t = idxp.tile([128, IDX_COLS], I16)
                nc.sync.dma_start(
                    out=idx_t[:], in_=eidx[:, sb * IDX_COLS : (sb + 1) * IDX_COLS]
                )
                meta_t = metap.tile([128, META_COLS], F32)
                nc.sync.dma_start(
                    out=meta_t[:], in_=meta[:, sb * META_COLS : (sb + 1) * META_COLS]
                )
                val_s = meta_t[:, 0:COLS]
                ldst_s = meta_t[:, COLS : 2 * COLS]
                coef_s = meta_t[:, 2 * COLS : META_COLS]

                # ---- gather: one dma_gather per src segment
                x_t = xg.tile([128, COLS, F], F32, tag="x")
                for s in range(NSEG):
                    nc.gpsimd.dma_gather(
                        out_ap=x_t[:, s * CS : (s + 1) * CS, :],
                        in_ap=tbl[s * (SEG + 1) :, :],
                        idxs_ap=idx_t[
                            :, s * (SEG_IDX // 16) : (s + 1) * (SEG_IDX // 16)
                        ],
                        num_idxs=SEG_IDX,
                        num_idxs_reg=SEG_IDX,
                        elem_size=F,
                        single_packet=False,
                    )

                # w4[e, col, bb] = val * coeff[r_e, bb]
                w4_t = w4p.tile([128, COLS * NB], F32)
                nc.vector.tensor_mul(
                    w4_t[:].rearrange("p (c b) -> p c b", b=NB),
                    val_s[:, :, None].to_broadcast([128, COLS, NB]),
                    coef_s.rearrange("p (c b) -> p c b", b=NB),
                )

                gt_ps = [
                    psg.tile([F, GPB * NB * GROUP], F32, tag="g", name=f"gt{b}")
                    for b in range(BPS)
                ]

                # region (b, q) chunk order: s-major; first chunk at s=0 is the
                # K=128 chunk, last at s=3 is the K=64 half.
                for s in range(NSEG):
                    for cis in range(CS):
                        col = s * CS + cis
                        # start=True arms a pending-zero for the WHOLE 2KB
                        # bank on trn2, so it must be issued exactly once per
                        # block bank (first matmul), never per q-region.
                        if cis < BUCKETS:
                            m_t = mskp.tile([128, NB * GROUP], F32, tag="m")
                            nc.vector.scalar_tensor_tensor(
                                out=m_t[:].rearrange("p (b n) -> p b n", b=NB),
                                in0=iota_t[:].rearrange("p (b n) -> p b n", b=NB),
                                scalar=ldst_s[:, col : col + 1],
                                in1=w4_t[:, col * NB : (col + 1) * NB][
                                    :, :, None
                                ].to_broadcast([128, NB, GROUP]),
                                op0=mybir.AluOpType.is_equal,
                                op1=mybir.AluOpType.mult,
                            )
                            bq = cis
                            b, q = bq // GPB, bq % GPB
                            nc.tensor.matmul(
                                gt_ps[b][:, q * 128 : (q + 1) * 128],
                                lhsT=x_t[:, col, :],
                                rhs=m_t[:],
                                start=(s == 0 and q == 0),
                                stop=False,
                                skip_group_check=True,
                            )
                        else:
                            # tail pair: buckets (2k, 2k+1) share block b,
                            # q regions (q0, q0+1); one K=128 N=256 matmul
                            # with a block-diagonal mask (iota2 sentinel
                            # zeroes the foreign partition half).
                            k = cis - BUCKETS
                            b, q0 = k // 2, (k % 2) * 2
                            m2_t = mskp.tile([128, 2 * NB * GROUP], F32, tag="m2")
                            half_cols = NB * GROUP
                            for qh in range(2):
                                nc.vector.scalar_tensor_tensor(
                                    out=m2_t[
                                        :, qh * half_cols : (qh + 1) * half_cols
                                    ].rearrange("p (b n) -> p b n", b=NB),
                                    in0=iota2_t[
                                        :, qh * half_cols : (qh + 1) * half_cols
                                    ].rearrange("p (b n) -> p b n", b=NB),
                                    scalar=ldst_s[:, col : col + 1],
                                    in1=w4_t[:, col * NB : (col + 1) * NB][
                                        :, :, None
                                    ].to_broadcast([128, NB, GROUP]),
                                    op0=mybir.AluOpType.is_equal,
                                    op1=mybir.AluOpType.mult,
                                )
                            nc.tensor.matmul(
                                gt_ps[b][:, q0 * 128 : (q0 + 2) * 128],
                                lhsT=x_t[:, col, :],
                                rhs=m2_t[:],
                                start=False,
                                stop=(s == NSEG - 1 and k % 2 == 1),
                                skip_group_check=True,
                            )

                # ---- per block: basis application + bias + store
                for b in range(BPS):
                    j = sb * BPS + b
                    gt_sb = gtp.tile([F, GPB * NB * GROUP], F32)
                    nc.scalar.copy(gt_sb[:], gt_ps[b][:])
                    ot_ps = pso.tile([F, BLOCK], F32)
                    gt_v = gt_sb[:].rearrange("p (q b n) -> p q b n", q=GPB, b=NB)
                    for bb in range(NB):
                        nc.tensor.matmul(
                            ot_ps[:].rearrange("p (q n) -> p q n", q=GPB),
                            lhsT=basis_t[:, bb * F : (bb + 1) * F],
                            rhs=gt_v[:, :, bb, :],
                            start=(bb == 0),
                            stop=(bb == NB - 1),
                        )
                    ot_sb = otp.tile([F, BLOCK], F32)
                    nc.scalar.activation(
                        ot_sb[:],
                        ot_ps[:],
                        mybir.ActivationFunctionType.Identity,
                        bias=bias_col[:],
                    )
                    nc.sync.dma_start(out=outT[j, :, :], in_=ot_sb[:])

    nc.compile()
    return nc


def _preprocess(basis_coeff, edge_val, edge_src, edge_dst):
    """Pack edges into the static (SB, segment, bucket, chunk) structure.
    Returns per-core (eidx [128, NSB*IDX_COLS] int16,
    meta [128, NSB*META_COLS] f32)."""
    src = np.ascontiguousarray(edge_src).ravel()
    dst = np.ascontiguousarray(edge_dst).ravel()
    val = np.ascontiguousarray(edge_val).ravel().astype(np.float32)
    rel = np.repeat(np.arange(NREL, dtype=np.int32), edge_src.shape[1])
    coeff = np.asarray(basis_coeff, dtype=np.float32)  # [NREL, NB]

    core = dst // NS
    per_core = []
    n_grp = NBLK * GPB  # 400 padded group slots (391 real)
    for c in range(NCORES):
        msel = core == c
        s_ = src[msel]
        dl = dst[msel] - c * NS
        v = val[msel]
        r = rel[msel]

        g = dl // GROUP                  # group 0..390
        w = (dl % GROUP).astype(np.float32)
        seg = s_ // SEG                  # 0..3
        lidx = (s_ % SEG + 1).astype(np.int16)  # 1..25000 (0 = zero row)

        bucket = g.astype(np.int64) * NSEG + seg
        order = np.argsort(bucket, kind="stable")
        s_, dl, v, r, g, w, seg, lidx, bucket = (
            a[order] for a in (s_, dl, v, r, g, w, seg, lidx, bucket)
        )
        cnt = np.bincount(bucket, minlength=n_grp * NSEG)
        assert cnt.max() <= CAP, f"bucket capacity exceeded: {cnt.max()} > {CAP}"
        starts = np.zeros(n_grp * NSEG + 1, dtype=np.int64)
        np.cumsum(cnt, out=starts[1:])
        pos = np.arange(len(s_)) - starts[bucket]

        # static slot map: (block j, q, seg, pos) -> (SB, X column, partition)
        j = g // GPB
        q = g % GPB
        sbi = j // BPS
        bis = (j % BPS) * GPB + q        # bucket index within (SB, seg), 0..15
        in128 = pos < 128
        cis = np.where(in128, bis, BUCKETS + bis // 2)
        part = np.where(in128, pos, (bis % 2) * 64 + (pos - 128))
        col = seg * CS + cis             # X column within SB, 0..95

        # gather position within (SB, seg): i = cis*128 + part
        gpos = cis * 128 + part

        # ---- index array: per (SB, seg) wrapped int16 [16, 192] tiled to 128
        idx_flat = np.zeros((NSB, NSEG, SEG_IDX), dtype=np.int16)
        idx_flat[sbi, seg, gpos] = lidx
        # wrap: position i = s16*16 + p16 -> [16, SEG_IDX//16]
        wrapped = idx_flat.reshape(NSB, NSEG, SEG_IDX // 16, 16).transpose(0, 1, 3, 2)
        # [NSB, NSEG, 16, 192] -> tile 16-partition pattern to 128 partitions
        wrapped = np.broadcast_to(
            wrapped[:, :, None, :, :], (NSB, NSEG, 8, 16, SEG_IDX // 16)
        ).reshape(NSB, NSEG, 128, SEG_IDX // 16)
        eidx_c = np.ascontiguousarray(
            wrapped.transpose(2, 0, 1, 3).reshape(128, NSB * IDX_COLS)
        )

        # ---- meta arrays [NSB, 128, META_COLS]
        mval = np.zeros((NSB, 128, COLS), dtype=np.float32)
        mldst = np.zeros((NSB, 128, COLS), dtype=np.float32)
        mcoef = np.zeros((NSB, 128, COLS, NB), dtype=np.float32)
        mval[sbi, part, col] = v
        mldst[sbi, part, col] = w
        mcoef[sbi, part, col] = coeff[r]
        meta_c = np.concatenate(
            [mval, mldst, mcoef.reshape(NSB, 128, COLS * NB)], axis=2
        )
        meta_c = np.ascontiguousarray(
            meta_c.transpose(1, 0, 2).reshape(128, NSB * META_COLS)
        )
        per_core.append((eidx_c, meta_c))
    return per_core


def _build_iota2():
    io2 = np.full((128, 2 * NB * GROUP), 99.0, dtype=np.float32)
    n_pat = np.tile(np.arange(GROUP, dtype=np.float32), NB)  # (bb, n) -> n
    io2[:64, :NB * GROUP] = n_pat[None, :]
    io2[64:, NB * GROUP :] = n_pat[None, :]
    return np.ascontiguousarray(io2)


def _build_table(inp):
    tbl = np.zeros((TBL_ROWS, F), dtype=np.float32)
    for s in range(NSEG):
        tbl[s * (SEG + 1) + 1 : (s + 1) * (SEG + 1)] = inp[s * SEG : (s + 1) * SEG]
    return tbl


def kernel(inp, basis_weights, basis_coeff, bias, edge_val, edge_src, edge_dst):
    inp = np.ascontiguousarray(np.asarray(inp, dtype=np.float32))
    basis_weights = np.ascontiguousarray(np.asarray(basis_weights, dtype=np.float32))
    basis_coeff = np.asarray(basis_coeff, dtype=np.float32)
    bias = np.ascontiguousarray(np.asarray(bias, dtype=np.float32))

    if "nc" not in _compiled:
        _compiled["nc"] = _build_program()
    nc = _compiled["nc"]

    per_core = _preprocess(basis_coeff, edge_val, edge_src, edge_dst)
    tbl = _build_table(inp)
    iota_np = np.ascontiguousarray(
        np.tile(np.arange(GROUP, dtype=np.float32), NB)[None, :].repeat(128, 0)
    )
    iota2_np = _build_iota2()

    in_maps = []
    for c in range(NCORES):
        eidx_c, meta_c = per_core[c]
        in_maps.append(
            {
                "tbl": tbl,
                "basisw": basis_weights,
                "biasw": bias,
                "iota": iota_np,
                "iota2": iota2_np,
                "eidx": eidx_c,
                "meta": meta_c,
            }
        )

    res = run_bass_kernel_spmd(nc, in_maps, list(range(NCORES)))
    _compiled["last_results"] = res

    out = np.empty((NN, F), dtype=np.float32)
    for c in range(NCORES):
        oT = res.results[c]["outT"]  # [NBLK, F, BLOCK]
        rows = oT.transpose(0, 2, 1).reshape(NBLK * BLOCK, F)[:NS]
        out[c * NS : (c + 1) * NS] = rows
    return out



# revision 4
# speedup vs baseline: 2.9635x; 2.9635x over previous
"""Trainium2 Bass kernel for an R-GCN-style GCN layer (basis decomposition).

Reference computation (per relation r, with W_r = sum_b coeff[r,b] * basis[b]):
    out = sum_r segment_sum(inp[src_r] * val_r, dst_r) @ W_r + sum_r bias[r]

Algebraic restructure (4 basis accumulators instead of 16 relation matmuls):
    out[d] = sum_b G_b[d] @ basis[b] + bias_sum
    G_b[d] = sum_{edges e: dst_e = d} (coeff[r_e, b] * val_e) * inp[src_e]

Distribution: output nodes are sharded 8 ways (12500 rows/core); every core
holds the full gather table in its own HBM -> no cross-core communication.

Key performance structure (vs the naive single-queue fp32 version):
  - The per-edge feature gather runs as dma_gather on 4 SWDGE queues
    (queue q is served by GPSIMD Q7 core pair (2q, 2q+1)), one queue per
    src segment, so descriptor generation runs 4-way parallel.
  - Whole datapath in bf16: gather table, masks, matmuls (PSUM stays fp32).
  - 20 X columns per (SB, seg): 16 base bucket columns (one per
    (block-in-SB, 32-dst-group)) plus 4 per-block shared overflow ("quad")
    columns whose mask spans the whole 512-wide PSUM bank.
  - Trailing padding slots get idx = -1, which dma_gather skips entirely.
  - Masks are built with 2 DVE ops per (SB, seg) over 32 "virtual columns"
    (16 base + 16 quad quarters):
        D[p, vc, n]      = iota[n] - ldst[p, vc]          (tensor_sub)
        M[p, vc, b, n]   = (D == 0) * w4[p, vc, b]        (one fused STT)
    with ldst/w4 packed per virtual column on the host.

Per-core shapes are identical across cores (SPMD); only data differs.
Output is produced transposed per block ([fout, node]) and reassembled on host.
"""
import os
import sys

for _p in ("/opt/trn_rl_repo", "/root/.axon_site/_ro/trn_rl_repo"):
    if os.path.isdir(_p) and _p not in sys.path:
        sys.path.insert(0, _p)

import ml_dtypes
import numpy as np

import concourse.bass as bass
import concourse.tile as tile
from concourse import bacc, mybir
from concourse.bass_utils import run_bass_kernel_spmd

# ---------------- problem constants (hardcoded from spec) ----------------
NN = 100000          # nodes
F = 128              # feature dim (in == out)
NB = 4               # bases
NREL = 16            # relations
NCORES = 8
NS = NN // NCORES    # dst nodes per core (12500)

GROUP = 32           # dst nodes per group
GPB = 4              # groups per block
BLOCK = GROUP * GPB  # 128 dst nodes per block
NBLK = 100           # padded block count (98 real)
BPS = 4              # blocks per superblock
NSB = NBLK // BPS    # 25 superblocks

NSEG = 4             # src segments
SEG = 25000          # src rows per segment
TBL_ROWS = NN + NSEG # composite table: one zero row per segment

CS = 20              # X columns per (SB, seg): 16 base + 4 quad overflow
SEG_IDX = CS * 128   # 2560 gather rows per (SB, segment)
COLS = NSEG * CS     # 80 X columns per SB

CV = 32              # virtual mask columns per (SB, seg): 16 base + 16 quad quarters
MSEG = CV + CV * NB  # meta cols per (SB, seg): ldst[32] + w4[128] (bf16)
META_COLS = NSEG * MSEG            # 640 per SB
IDX_COLS = NSEG * (SEG_IDX // 16)  # 640 int16 cols per SB

F32 = mybir.dt.float32
BF16 = mybir.dt.bfloat16
I16 = mybir.dt.int16
NPBF16 = ml_dtypes.bfloat16

_compiled = {}


def _build_program():
    nc = bacc.Bacc(
        "TRN2",
        target_bir_lowering=False,
        debug=False,
        enable_asserts=False,
        num_devices=NCORES,
        num_swdge_queues=4,
    )

    tbl = nc.dram_tensor("tbl", [TBL_ROWS, F], BF16, kind="ExternalInput")
    basisw = nc.dram_tensor("basisw", [NB, F, F], BF16, kind="ExternalInput")
    biasw = nc.dram_tensor("biasw", [NREL, F], F32, kind="ExternalInput")
    iota = nc.dram_tensor("iota", [128, GROUP], BF16, kind="ExternalInput")
    eidx = nc.dram_tensor("eidx", [128, NSB * IDX_COLS], I16, kind="ExternalInput")
    meta = nc.dram_tensor("meta", [128, NSB * META_COLS], BF16, kind="ExternalInput")
    outT = nc.dram_tensor("outT", [NBLK, F, BLOCK], F32, kind="ExternalOutput")

    with tile.TileContext(nc) as tc:
        with (
            tc.tile_pool(name="const", bufs=1) as const,
            tc.tile_pool(name="xg", bufs=3) as xg,
            tc.tile_pool(name="idxp", bufs=2) as idxp,
            tc.tile_pool(name="metap", bufs=2) as metap,
            tc.tile_pool(name="dp", bufs=4) as dp,
            tc.tile_pool(name="msk", bufs=5) as mskp,
            tc.tile_pool(name="gt", bufs=4) as gtp,
            tc.tile_pool(name="ot", bufs=3) as otp,
            tc.tile_pool(name="psg", bufs=5, space="PSUM") as psg,
            tc.tile_pool(name="pso", bufs=2, space="PSUM") as pso,
            tc.tile_pool(name="psb", bufs=1, space="PSUM") as psb,
        ):
            # ---- constants
            iota_t = const.tile([128, GROUP], BF16)
            nc.sync.dma_start(out=iota_t[:], in_=iota[:, :])
            basis_t = const.tile([F, NB * F], BF16)
            for b in range(NB):
                nc.sync.dma_start(
                    out=basis_t[:, b * F : (b + 1) * F], in_=basisw[b, :, :]
                )
            bias_sb = const.tile([NREL, F], F32)
            nc.sync.dma_start(out=bias_sb[:], in_=biasw[:, :])
            ones_t = const.tile([NREL, 1], F32)
            nc.vector.memset(ones_t[:], 1.0)
            bias_ps = psb.tile([F, 1], F32)
            nc.tensor.matmul(
                bias_ps[:], lhsT=bias_sb[:], rhs=ones_t[:], start=True, stop=True
            )
            bias_col = const.tile([F, 1], F32)
            nc.scalar.copy(bias_col[:], bias_ps[:])

            for sb in range(NSB):
                idx_t = idxp.tile([128, IDX_COLS], I16)
                nc.sync.dma_start(
                    out=idx_t[:], in_=eidx[:, sb * IDX_COLS : (sb + 1) * IDX_COLS]
                )
                meta_t = metap.tile([128, META_COLS], BF16)
                nc.sync.dma_start(
                    out=meta_t[:], in_=meta[:, sb * META_COLS : (sb + 1) * META_COLS]
                )

                x_t = xg.tile([128, COLS, F], BF16, tag="x")
                if sb < 3:
                    # first use of each ring buffer: clear so that slots
                    # skipped by the trailing -1 trim never read NaN garbage
                    nc.vector.memset(x_t[:], 0.0)

                # ---- gather: one dma_gather per src segment, one SWDGE
                # queue (= Q7 core pair) per segment -> 4-way parallel DGE
                for s in range(NSEG):
                    nc.gpsimd.dma_gather(
                        out_ap=x_t[:, s * CS : (s + 1) * CS, :],
                        in_ap=tbl[s * (SEG + 1) :, :],
                        idxs_ap=idx_t[
                            :, s * (SEG_IDX // 16) : (s + 1) * (SEG_IDX // 16)
                        ],
                        num_idxs=SEG_IDX,
                        num_idxs_reg=SEG_IDX,
                        elem_size=F,
                        single_packet=False,
                        queue_num=s,
                    )

                gt_ps = [
                    psg.tile([F, GPB * NB * GROUP], F32, tag="g", name=f"gt{b}")
                    for b in range(BPS)
                ]

                for s in range(NSEG):
                    ldst_s = meta_t[:, s * MSEG : s * MSEG + CV]
                    w4_s = meta_t[:, s * MSEG + CV : (s + 1) * MSEG]

                    # D[p, vc, n] = iota[n] - ldst[p, vc]
                    d_t = dp.tile([128, CV * GROUP], BF16, tag="d")
                    nc.vector.tensor_sub(
                        d_t[:].rearrange("p (c n) -> p c n", n=GROUP),
                        iota_t[:][:, None, :].to_broadcast([128, CV, GROUP]),
                        ldst_s[:, :, None].to_broadcast([128, CV, GROUP]),
                    )
                    # M[p, vc, b, n] = (D == 0) * w4[p, vc, b]
                    # (one STT per basis b: walrus only allows 2-3D APs)
                    m_t = mskp.tile([128, CV * NB * GROUP], BF16, tag="m")
                    m_v = m_t[:].rearrange("p (c b n) -> p c b n", b=NB, n=GROUP)
                    d_v = d_t[:].rearrange("p (c n) -> p c n", n=GROUP)
                    w4_v = w4_s.rearrange("p (c b) -> p c b", b=NB)
                    for bb in range(NB):
                        nc.vector.scalar_tensor_tensor(
                            out=m_v[:, :, bb, :],
                            in0=d_v,
                            scalar=0.0,
                            in1=w4_v[:, :, bb : bb + 1].to_broadcast(
                                [128, CV, GROUP]
                            ),
                            op0=mybir.AluOpType.is_equal,
                            op1=mybir.AluOpType.mult,
                        )

                    # ---- per-column matmuls into the 4 block PSUM banks.
                    # start=True arms a pending-zero for the WHOLE 2KB bank on
                    # trn2, so it is issued exactly once per bank (first base
                    # column of the bank at s=0); stop on the s=3 quad column.
                    for col in range(CS):
                        xcol = x_t[:, s * CS + col, :]
                        if col < 16:
                            b, q = col // GPB, col % GPB
                            nc.tensor.matmul(
                                gt_ps[b][:, q * 128 : (q + 1) * 128],
                                lhsT=xcol,
                                rhs=m_t[:, col * 128 : (col + 1) * 128],
                                start=(s == 0 and q == 0),
                                stop=False,
                                skip_group_check=True,
                            )
                        else:
                            b = col - 16
                            v0 = 16 + 4 * b
                            nc.tensor.matmul(
                                gt_ps[b][:, 0 : 4 * 128],
                                lhsT=xcol,
                                rhs=m_t[:, v0 * 128 : (v0 + 4) * 128],
                                start=False,
                                stop=(s == NSEG - 1),
                                skip_group_check=True,
                            )

                # ---- per block: basis application + bias + store
                for b in range(BPS):
                    j = sb * BPS + b
                    gt_sb = gtp.tile([F, GPB * NB * GROUP], BF16)
                    nc.scalar.copy(gt_sb[:], gt_ps[b][:])
                    ot_ps = pso.tile([F, BLOCK], F32)
                    gt_v = gt_sb[:].rearrange("p (q b n) -> p q b n", q=GPB, b=NB)
                    for bb in range(NB):
                        nc.tensor.matmul(
                            ot_ps[:].rearrange("p (q n) -> p q n", q=GPB),
                            lhsT=basis_t[:, bb * F : (bb + 1) * F],
                            rhs=gt_v[:, :, bb, :],
                            start=(bb == 0),
                            stop=(bb == NB - 1),
                        )
                    ot_sb = otp.tile([F, BLOCK], F32)
                    nc.scalar.activation(
                        ot_sb[:],
                        ot_ps[:],
                        mybir.ActivationFunctionType.Identity,
                        bias=bias_col[:],
                    )
                    nc.sync.dma_start(out=outT[j, :, :], in_=ot_sb[:])

    nc.compile()
    return nc


def _preprocess(basis_coeff, edge_val, edge_src, edge_dst):
    """Pack edges into the static (SB, segment, column) structure.
    Returns per-core (eidx [128, NSB*IDX_COLS] int16,
    meta [128, NSB*META_COLS] bf16)."""
    src = np.ascontiguousarray(edge_src).ravel()
    dst = np.ascontiguousarray(edge_dst).ravel()
    val = np.ascontiguousarray(edge_val).ravel().astype(np.float32)
    rel = np.repeat(np.arange(NREL, dtype=np.int32), edge_src.shape[1])
    coeff = np.asarray(basis_coeff, dtype=np.float32)  # [NREL, NB]

    core = dst // NS
    per_core = []
    n_grp = NBLK * GPB  # 400 padded group slots (391 real)
    for c in range(NCORES):
        msel = core == c
        s_ = src[msel]
        dl = dst[msel] - c * NS
        v = val[msel]
        r = rel[msel]

        g = dl // GROUP                  # group 0..390
        w = (dl % GROUP).astype(np.float32)
        seg = s_ // SEG                  # 0..3
        lidx = (s_ % SEG + 1).astype(np.int16)  # 1..25000 (0 = zero row)

        bucket = g.astype(np.int64) * NSEG + seg
        order = np.argsort(bucket, kind="stable")
        s_, dl, v, r, g, w, seg, lidx, bucket = (
            a[order] for a in (s_, dl, v, r, g, w, seg, lidx, bucket)
        )
        cnt = np.bincount(bucket, minlength=n_grp * NSEG)
        starts = np.zeros(n_grp * NSEG + 1, dtype=np.int64)
        np.cumsum(cnt, out=starts[1:])
        pos = np.arange(len(s_)) - starts[bucket]

        j = g // GPB                     # block 0..97
        q = g % GPB
        sbi = j // BPS
        b_in = j % BPS                   # block within SB
        bis = b_in * GPB + q             # base column index, 0..15

        in128 = pos < 128
        # overflow edges: position within the (block, seg) quad column,
        # ordered by (q, pos) -- the stable sort by bucket gives that order
        # once regrouped by (j, seg)
        ov_idx = np.nonzero(~in128)[0]
        ov_key = (j[ov_idx].astype(np.int64) * NSEG + seg[ov_idx])
        ov_order = np.argsort(ov_key, kind="stable")
        ov_sorted = ov_idx[ov_order]
        ov_key_sorted = ov_key[ov_order]
        ov_cnt = np.bincount(ov_key_sorted, minlength=NBLK * NSEG)
        assert ov_cnt.max() <= 128, f"quad overflow exceeded: {ov_cnt.max()}"
        ov_starts = np.zeros(NBLK * NSEG + 1, dtype=np.int64)
        np.cumsum(ov_cnt, out=ov_starts[1:])
        ovpos = np.zeros(len(s_), dtype=np.int64)
        ovpos[ov_sorted] = np.arange(len(ov_sorted)) - ov_starts[ov_key_sorted]

        col = np.where(in128, bis, 16 + b_in)          # physical col 0..19
        part = np.where(in128, pos, ovpos)
        vc = np.where(in128, bis, 16 + 4 * b_in + q)   # virtual col 0..31
        gpos = col * 128 + part

        # ---- index array (0 = per-segment zero row for padding slots)
        idx_flat = np.zeros((NSB, NSEG, SEG_IDX), dtype=np.int16)
        idx_flat[sbi, seg, gpos] = lidx

        # wrap: position i = s16*16 + p16 -> [16, SEG_IDX//16], tiled to 128
        wrapped = idx_flat.reshape(NSB, NSEG, SEG_IDX // 16, 16).transpose(0, 1, 3, 2)
        wrapped = np.broadcast_to(
            wrapped[:, :, None, :, :], (NSB, NSEG, 8, 16, SEG_IDX // 16)
        ).reshape(NSB, NSEG, 128, SEG_IDX // 16)
        eidx_c = np.ascontiguousarray(
            wrapped.transpose(2, 0, 1, 3).reshape(128, NSB * IDX_COLS)
        )

        # ---- meta arrays: per (SB, seg): [ldst: CV][w4: CV*NB], bf16
        mldst = np.zeros((NSB, NSEG, 128, CV), dtype=np.float32)
        mw4 = np.zeros((NSB, NSEG, 128, CV, NB), dtype=np.float32)
        mldst[sbi, seg, part, vc] = w
        mw4[sbi, seg, part, vc] = v[:, None] * coeff[r]
        meta_seg = np.concatenate(
            [mldst, mw4.reshape(NSB, NSEG, 128, CV * NB)], axis=3
        )  # [NSB, NSEG, 128, MSEG]
        meta_c = np.ascontiguousarray(
            meta_seg.transpose(2, 0, 1, 3).reshape(128, NSB * META_COLS)
        ).astype(NPBF16)
        per_core.append((eidx_c, meta_c))
    return per_core


def _build_table(inp):
    tbl = np.zeros((TBL_ROWS, F), dtype=np.float32)
    for s in range(NSEG):
        tbl[s * (SEG + 1) + 1 : (s + 1) * (SEG + 1)] = inp[s * SEG : (s + 1) * SEG]
    return tbl.astype(NPBF16)


def kernel(inp, basis_weights, basis_coeff, bias, edge_val, edge_src, edge_dst):
    inp = np.ascontiguousarray(np.asarray(inp, dtype=np.float32))
    basis_weights = np.ascontiguousarray(np.asarray(basis_weights, dtype=np.float32))
    basis_coeff = np.asarray(basis_coeff, dtype=np.float32)
    bias = np.ascontiguousarray(np.asarray(bias, dtype=np.float32))

    if "nc" not in _compiled:
        _compiled["nc"] = _build_program()
    nc = _compiled["nc"]

    per_core = _preprocess(basis_coeff, edge_val, edge_src, edge_dst)
    tbl = _build_table(inp)
    iota_np = np.ascontiguousarray(
        np.arange(GROUP, dtype=np.float32)[None, :].repeat(128, 0)
    ).astype(NPBF16)
    basis_bf = basis_weights.astype(NPBF16)

    in_maps = []
    for c in range(NCORES):
        eidx_c, meta_c = per_core[c]
        in_maps.append(
            {
                "tbl": tbl,
                "basisw": basis_bf,
                "biasw": bias,
                "iota": iota_np,
                "eidx": eidx_c,
                "meta": meta_c,
            }
        )

    res = run_bass_kernel_spmd(nc, in_maps, list(range(NCORES)))
    _compiled["last_results"] = res

    out = np.empty((NN, F), dtype=np.float32)
    for c in range(NCORES):
        oT = res.results[c]["outT"]  # [NBLK, F, BLOCK]
        rows = oT.transpose(0, 2, 1).reshape(NBLK * BLOCK, F)[:NS]
        out[c * NS : (c + 1) * NS] = rows
    return out
